# revision 15
# baseline (speedup 1.0000x reference)
"""Multi-head attention (B=8, N=1024, D=768, H=12) on 8 TRN2 NeuronCores.

Sharding: batch-parallel. Core i computes the full 12-head attention for
batch element i; weights are replicated. No collectives.

Per-core dataflow (all matmuls bf16 with fp32 PSUM accumulation):
  1. x [1024,768] f32 -> PE-transpose -> xT [768,1024] bf16 (k on partitions)
  2. v = x @ Wv + bv (natural layout [n, d], bias folded in during drain;
     P @ (v + 1 bv^T) / denom == P @ v / denom + bv exactly)
     qT = Wq^T x^T, kT = Wk^T x^T  ([d, n] layout, bias fused in DVE drain)
  3. per head pair t (heads 2t, 2t+1 live at partitions 0-63 / 64-127 of
     d-tile t):
       S^T[nk, nq] = kT^T @ qT   (K=64, two heads row-packed via base_partition)
       P^T = exp(S^T / 8)        (ACT, PSUM->SBUF bf16, scale fused)
       outU^T[d+1, nq] = [v | 1]^T @ P^T  (ones column yields softmax denoms)
       PE-transpose outU^T back to natural, multiply by 1/denom (per-partition
       scalar) into out_nat
     The qk projection of pair t+1 is emitted between S(t) and PV(t) so the
     in-order PE queue has independent work while ACT runs the exps.
  4. DMA out per n-tile.
"""

import threading

import numpy as np

import concourse.bass as bass
import concourse.tile as tile
from concourse import mybir
from concourse.bass_utils import run_bass_kernel_spmd
from concourse.masks import make_identity

B, N, D, H, HD = 8, 1024, 768, 12, 64
P = 128
NT = N // P          # 8  n-tiles
DT = D // P          # 6  d-tiles == head pairs
NC2 = N // 512       # 2  512-wide n chunks
HDE = HD + 1         # 65 head cols incl. ones column
FP32 = mybir.dt.float32
BF16 = mybir.dt.bfloat16
AF = mybir.ActivationFunctionType


def build_mha_bass(split_waits: bool = True) -> bass.Bass:
    nc = bass.Bass()

    x_d = nc.declare_dram_parameter("x", [N, D], FP32, isOutput=False)
    wq_d = nc.declare_dram_parameter("Wq", [D, D], FP32, isOutput=False)
    bq_d = nc.declare_dram_parameter("bq", [D], FP32, isOutput=False)
    wk_d = nc.declare_dram_parameter("Wk", [D, D], FP32, isOutput=False)
    bk_d = nc.declare_dram_parameter("bk", [D], FP32, isOutput=False)
    wv_d = nc.declare_dram_parameter("Wv", [D, D], FP32, isOutput=False)
    bv_d = nc.declare_dram_parameter("bv", [D], FP32, isOutput=False)
    out_d = nc.declare_dram_parameter("out", [N, D], FP32, isOutput=True)

    with tile.TileContext(nc) as tc:
        with tc.tile_pool(name="singles", bufs=1) as singles:
            ident = singles.tile([P, P], FP32)
            make_identity(nc, ident)

            # biases: bq/bk as [128, DT] (per-partition scalars per d-tile),
            # bv broadcast across partitions (folded into the v drain).
            bq_sb = singles.tile([P, DT], FP32)
            bk_sb = singles.tile([P, DT], FP32)
            bv_bc = singles.tile([P, D], FP32)

            w_bf = {}
            for wi in (2, 0, 1):
                for kt in range(DT):
                    w_bf[(wi, kt)] = singles.tile([P, D], BF16, name=f"wbf{wi}_{kt}")
            x_T = [singles.tile([P, N], BF16, name=f"xT_{c}") for c in range(DT)]
            v_ext = [singles.tile([P, H * HDE], BF16, name=f"vext_{j}") for j in range(NT)]
            for j in range(NT):
                ones_cols = v_ext[j].rearrange("p (h c) -> p h c", c=HDE)[:, :, HD:HDE]
                nc.vector.memset(ones_cols, 1.0)

            out_nat = [singles.tile([P, D], FP32, name=f"onat_{i}") for i in range(NT)]

            with (
                tc.tile_pool(name="xstage", bufs=NT) as xstage,
                tc.tile_pool(name="wstage", bufs=18) as wstage,
                tc.tile_pool(name="xt_ps", bufs=4, space="PSUM") as xtps,
                tc.tile_pool(name="v_ps", bufs=2, space="PSUM") as vps,
            ):
                # ---- input DMAs, latency-critical first: x, then Wv, Wq, Wk ----
                x_st = []
                for i in range(NT):
                    xst = xstage.tile([P, D], FP32, tag="xs", name=f"xst_{i}")
                    nc.gpsimd.dma_start(out=xst, in_=x_d[i * P:(i + 1) * P, :])
                    x_st.append(xst)

                w_st = {}
                for wi, wd in ((2, wv_d), (0, wq_d), (1, wk_d)):
                    for kt in range(DT):
                        wst = wstage.tile([P, D], FP32, tag="wst", name=f"wst{wi}_{kt}")
                        nc.gpsimd.dma_start(out=wst, in_=wd[kt * P:(kt + 1) * P, :])
                        w_st[(wi, kt)] = wst

                nc.gpsimd.dma_start(out=bq_sb, in_=bq_d[:].rearrange("(t p) -> p t", p=P))
                nc.gpsimd.dma_start(out=bk_sb, in_=bk_d[:].rearrange("(t p) -> p t", p=P))
                bv_ap = bv_d[:]
                nc.gpsimd.dma_start(
                    out=bv_bc,
                    in_=bass.AP(tensor=bv_ap.tensor, offset=bv_ap.offset, ap=[[0, P], [1, D]]),
                )

                # ---- weights -> bf16 (Wv first; DVE) ----
                for wi in (2, 0, 1):
                    for kt in range(DT):
                        nc.vector.tensor_copy(out=w_bf[(wi, kt)], in_=w_st[(wi, kt)])

                # ---- x -> xT (bf16) via PE transpose ----
                for i in range(NT):
                    for c in range(DT):
                        xps = xtps.tile([P, P], FP32, tag="xtps")
                        nc.tensor.transpose(xps, x_st[i][:, c * P:(c + 1) * P], ident)
                        nc.vector.tensor_copy(out=x_T[c][:, i * P:(i + 1) * P], in_=xps)

                # ---- v = x @ Wv + bv, drained with ones columns interleaved ----
                for j in range(NT):
                    pv = vps.tile([P, D], FP32, tag="vps")
                    for kt in range(DT):
                        lhs = x_T[kt][:, j * P:(j + 1) * P]
                        nc.tensor.matmul(
                            pv[:, 0:512], lhsT=lhs, rhs=w_bf[(2, kt)][:, 0:512],
                            start=(kt == 0), stop=(kt == DT - 1))
                        nc.tensor.matmul(
                            pv[:, 512:D], lhsT=lhs, rhs=w_bf[(2, kt)][:, 512:D],
                            start=(kt == 0), stop=(kt == DT - 1))
                    vdst = v_ext[j].rearrange("p (h c) -> p h c", c=HDE)[:, :, 0:HD]
                    vsrc = pv.rearrange("p (h c) -> p h c", c=HD)
                    bvv = bv_bc.rearrange("p (h c) -> p h c", c=HD)
                    nc.vector.tensor_add(vdst, vsrc, bvv)

            # ---- attention, pipelined over head pairs ----
            with (
                tc.tile_pool(name="qk_sb", bufs=2) as qkp,
                tc.tile_pool(name="p_sb", bufs=16) as pp,
                tc.tile_pool(name="ot_sb", bufs=3) as otp,
                tc.tile_pool(name="rc_sb", bufs=6) as rcp,
                tc.tile_pool(name="qk_ps", bufs=2, space="PSUM") as qkps,
                tc.tile_pool(name="s_ps", bufs=2, space="PSUM") as sps,
                tc.tile_pool(name="o_ps", bufs=1, space="PSUM") as ops,
            ):
                def qk_proj(t):
                    """q^T / k^T projection for d-tile t (bias fused, DVE drain)."""
                    qT = qkp.tile([P, N], BF16, tag="qT", name=f"qT_{t}")
                    kT = qkp.tile([P, N], BF16, tag="kT", name=f"kT_{t}")
                    for dst_sb, wi, b_sb in ((qT, 0, bq_sb), (kT, 1, bk_sb)):
                        for c in range(NC2):
                            qps = qkps.tile([P, 512], FP32, tag="qkps")
                            for kt in range(DT):
                                nc.tensor.matmul(
                                    qps,
                                    lhsT=w_bf[(wi, kt)][:, t * P:(t + 1) * P],
                                    rhs=x_T[kt][:, c * 512:(c + 1) * 512],
                                    start=(kt == 0), stop=(kt == DT - 1))
                            nc.vector.tensor_scalar_add(
                                dst_sb[:, c * 512:(c + 1) * 512], qps,
                                b_sb[:, t:t + 1])
                    return qT, kT

                qT, kT = qk_proj(0)
                for t in range(DT):
                    # --- S^T = k^T.T @ q^T (row-packed head pair) + exp ---
                    p_tiles = {}
                    for j in range(NT):
                        for c in range(NC2):
                            s_ps = sps.tile([P, 1024], FP32, tag="sps")
                            for h in range(2):
                                nc.tensor.matmul(
                                    s_ps[:, h * 512:(h + 1) * 512],
                                    lhsT=kT[h * HD:(h + 1) * HD, j * P:(j + 1) * P],
                                    rhs=qT[h * HD:(h + 1) * HD, c * 512:(c + 1) * 512],
                                    start=True, stop=True)
                            p_t = pp.tile([P, 1024], BF16, tag="p", name=f"p_{j}_{c}")
                            nc.scalar.activation(
                                out=p_t, in_=s_ps, func=AF.Exp,
                                bias=0.0, scale=0.125)
                            p_tiles[(j, c)] = p_t

                    # --- next pair's projection: independent PE work that
                    # fills the in-order PE queue while ACT runs the exps ---
                    if t + 1 < DT:
                        next_qT, next_kT = qk_proj(t + 1)
                    else:
                        next_qT = next_kT = None

                    # --- out^T_ext = [v | 1]^T @ P^T, one head at a time ---
                    for h in range(2):
                        gh = 2 * t + h
                        po = ops.tile([HDE, N], FP32, tag="ops", name=f"po_{t}_{h}")
                        for j in range(NT):
                            vl = v_ext[j][:, gh * HDE:(gh + 1) * HDE]
                            for c in range(NC2):
                                nc.tensor.matmul(
                                    po[:, c * 512:(c + 1) * 512],
                                    lhsT=vl,
                                    rhs=p_tiles[(j, c)][:, h * 512:(h + 1) * 512],
                                    start=(j == 0), stop=(j == NT - 1),
                                    skip_group_check=True)
                        ot = otp.tile([HDE, N], FP32, tag="ot", name=f"ot_{t}_{h}")
                        nc.vector.tensor_copy(out=ot, in_=po)
                        # transpose back to natural + normalize by denominators
                        for i in range(NT):
                            tps = qkps.tile([P, HDE], FP32, tag="qkps")
                            nc.tensor.transpose(
                                tps, ot[:, i * P:(i + 1) * P], ident[0:HDE, 0:HDE])
                            rc = rcp.tile([P, 1], FP32, tag="rc")
                            nc.vector.reciprocal(rc, tps[:, HD:HDE])
                            nc.vector.tensor_scalar_mul(
                                out_nat[i][:, gh * HD:(gh + 1) * HD],
                                tps[:, 0:HD], rc)
                            if t == DT - 1 and h == 1:
                                nc.gpsimd.dma_start(
                                    out=out_d[i * P:(i + 1) * P, :], in_=out_nat[i])

                    qT, kT = next_qT, next_kT

    if split_waits:
        _split_dma_waits(nc)
    return nc


_MAX_INLINE_WAITS = {"InstDMACopy": 1}
_DEFAULT_MAX_WAITS = 1


def _split_dma_waits(nc: bass.Bass) -> None:
    """walrus's instruction templates have limited semaphore-wait slots (one
    for the static-DMA pseudo, one for matmul's LDWEIGHTS, etc). Tile's sem
    assignment can attach more; hoist the excess into standalone
    InstEventSemaphore instructions on the issuing engine (sequencers execute
    in order, so the semantics are identical)."""
    for f in nc.m.functions:
        for bb in f.blocks:
            new_insts = []
            for inst in bb.instructions:
                si = getattr(inst, "sync_info", None)
                cap = _MAX_INLINE_WAITS.get(type(inst).__name__, _DEFAULT_MAX_WAITS)
                if si is not None and si.on_wait and len(si.on_wait) > cap:
                    waits = list(si.on_wait)
                    for wi, w in enumerate(waits[:-cap]):
                        ev = mybir.InstEventSemaphore(
                            name=f"{inst.name}-prewait{wi}")
                        ev.engine = inst.engine
                        ev.sync_info = mybir.SyncInfo(on_wait=[w], on_update=[])
                        new_insts.append(ev)
                    si.on_wait = waits[-cap:]
                    inst.sync_info = si
                new_insts.append(inst)
            bb.instructions[:] = new_insts


_BUILD_LOCK = threading.Lock()
_NC_CACHE: list = []


def _get_nc() -> bass.Bass:
    with _BUILD_LOCK:
        if not _NC_CACHE:
            _NC_CACHE.append(build_mha_bass())
    return _NC_CACHE[0]


def kernel(x, Wq, bq, Wk, bk, Wv, bv):
    x = np.ascontiguousarray(np.asarray(x, dtype=np.float32))
    Wq = np.ascontiguousarray(np.asarray(Wq, dtype=np.float32))
    Wk = np.ascontiguousarray(np.asarray(Wk, dtype=np.float32))
    Wv = np.ascontiguousarray(np.asarray(Wv, dtype=np.float32))
    bq = np.ascontiguousarray(np.asarray(bq, dtype=np.float32))
    bk = np.ascontiguousarray(np.asarray(bk, dtype=np.float32))
    bv = np.ascontiguousarray(np.asarray(bv, dtype=np.float32))
    assert x.shape == (B, N, D), x.shape

    nc = _get_nc()
    in_maps = [
        {"x": x[i], "Wq": Wq, "bq": bq, "Wk": Wk, "bk": bk, "Wv": Wv, "bv": bv}
        for i in range(B)
    ]
    res = run_bass_kernel_spmd(nc, in_maps, core_ids=list(range(B)))
    return np.stack([res.results[i]["out"] for i in range(B)], axis=0)


# revision 18
# speedup vs baseline: 1.0665x; 1.0665x over previous
"""Multi-head attention (B=8, N=1024, D=768, H=12) on 8 TRN2 NeuronCores.

Sharding: batch-parallel. Core i computes the full 12-head attention for
batch element i; weights are replicated. No collectives.

Per-core dataflow (all matmuls bf16 with fp32 PSUM accumulation):
  1. x [1024,768] f32 -> PE-transpose -> xT [768,1024] bf16 (k on partitions)
  2. v = x @ Wv + bv (natural layout [n, d], bias folded in during drain;
     P @ (v + 1 bv^T) / denom == P @ v / denom + bv exactly)
     qT = Wq^T x^T, kT = Wk^T x^T  ([d, n] layout, bias fused in DVE drain)
  3. per head pair t (heads 2t, 2t+1 live at partitions 0-63 / 64-127 of
     d-tile t):
       S^T[nk, nq] = kT^T @ qT   (K=64, two heads row-packed via base_partition)
       P^T = exp(S^T / 8)        (ACT, PSUM->SBUF bf16, scale fused)
       outU^T[d+1, nq] = [v | 1]^T @ P^T  (ones column yields softmax denoms)
       PE-transpose outU^T back to natural, multiply by 1/denom (per-partition
       scalar) into out_nat
     The qk projection of pair t+1 is emitted between S(t) and PV(t) so the
     in-order PE queue has independent work while ACT runs the exps.
  4. DMA out per n-tile.
"""

import threading

import numpy as np

import concourse.bass as bass
import concourse.tile as tile
from concourse import mybir
from concourse.bass_utils import run_bass_kernel_spmd
from concourse.masks import make_identity

B, N, D, H, HD = 8, 1024, 768, 12, 64
P = 128
NT = N // P          # 8  n-tiles
DT = D // P          # 6  d-tiles == head pairs
NC2 = N // 512       # 2  512-wide n chunks
HDE = HD + 1         # 65 head cols incl. ones column
FP32 = mybir.dt.float32
BF16 = mybir.dt.bfloat16
AF = mybir.ActivationFunctionType


def build_mha_bass(split_waits: bool = True) -> bass.Bass:
    nc = bass.Bass()

    x_d = nc.declare_dram_parameter("x", [N, D], FP32, isOutput=False)
    wq_d = nc.declare_dram_parameter("Wq", [D, D], FP32, isOutput=False)
    bq_d = nc.declare_dram_parameter("bq", [D], FP32, isOutput=False)
    wk_d = nc.declare_dram_parameter("Wk", [D, D], FP32, isOutput=False)
    bk_d = nc.declare_dram_parameter("bk", [D], FP32, isOutput=False)
    wv_d = nc.declare_dram_parameter("Wv", [D, D], FP32, isOutput=False)
    bv_d = nc.declare_dram_parameter("bv", [D], FP32, isOutput=False)
    out_d = nc.declare_dram_parameter("out", [N, D], FP32, isOutput=True)

    with tile.TileContext(nc) as tc:
        with tc.tile_pool(name="singles", bufs=1) as singles:
            ident = singles.tile([P, P], FP32)
            make_identity(nc, ident)

            # biases: bq/bk as [128, DT] (per-partition scalars per d-tile),
            # bv broadcast across partitions (folded into the v drain).
            bq_sb = singles.tile([P, DT], FP32)
            bk_sb = singles.tile([P, DT], FP32)
            bv_bc = singles.tile([P, D], FP32)

            w_bf = {}
            for wi in (2, 0, 1):
                for kt in range(DT):
                    w_bf[(wi, kt)] = singles.tile([P, D], BF16, name=f"wbf{wi}_{kt}")
            x_T = [singles.tile([P, N], BF16, name=f"xT_{c}") for c in range(DT)]
            v_ext = [singles.tile([P, H * HDE], BF16, name=f"vext_{j}") for j in range(NT)]
            for j in range(NT):
                ones_cols = v_ext[j].rearrange("p (h c) -> p h c", c=HDE)[:, :, HD:HDE]
                nc.vector.memset(ones_cols, 1.0)

            out_nat = [singles.tile([P, D], FP32, name=f"onat_{i}") for i in range(NT)]

            with (
                tc.tile_pool(name="xstage", bufs=NT) as xstage,
                tc.tile_pool(name="wstage", bufs=18) as wstage,
                tc.tile_pool(name="xt_ps", bufs=4, space="PSUM") as xtps,
                tc.tile_pool(name="v_ps", bufs=2, space="PSUM") as vps,
            ):
                # ---- input DMAs, latency-critical first: x, then Wv, Wq, Wk ----
                x_st = []
                for i in range(NT):
                    xst = xstage.tile([P, D], FP32, tag="xs", name=f"xst_{i}")
                    nc.gpsimd.dma_start(out=xst, in_=x_d[i * P:(i + 1) * P, :])
                    x_st.append(xst)

                w_st = {}
                for wi, wd in ((2, wv_d), (0, wq_d), (1, wk_d)):
                    for kt in range(DT):
                        wst = wstage.tile([P, D], FP32, tag="wst", name=f"wst{wi}_{kt}")
                        nc.gpsimd.dma_start(out=wst, in_=wd[kt * P:(kt + 1) * P, :])
                        w_st[(wi, kt)] = wst

                nc.gpsimd.dma_start(out=bq_sb, in_=bq_d[:].rearrange("(t p) -> p t", p=P))
                nc.gpsimd.dma_start(out=bk_sb, in_=bk_d[:].rearrange("(t p) -> p t", p=P))
                bv_ap = bv_d[:]
                nc.gpsimd.dma_start(
                    out=bv_bc,
                    in_=bass.AP(tensor=bv_ap.tensor, offset=bv_ap.offset, ap=[[0, P], [1, D]]),
                )

                # ---- weights -> bf16 (Wv first; DVE) ----
                for wi in (2, 0, 1):
                    for kt in range(DT):
                        nc.vector.tensor_copy(out=w_bf[(wi, kt)], in_=w_st[(wi, kt)])

                # ---- x -> xT (bf16) via PE transpose ----
                for i in range(NT):
                    for c in range(DT):
                        xps = xtps.tile([P, P], FP32, tag="xtps")
                        nc.tensor.transpose(xps, x_st[i][:, c * P:(c + 1) * P], ident)
                        nc.vector.tensor_copy(out=x_T[c][:, i * P:(i + 1) * P], in_=xps)

                # ---- v = x @ Wv + bv, drained with ones columns interleaved ----
                for j in range(NT):
                    pv = vps.tile([P, D], FP32, tag="vps")
                    for kt in range(DT):
                        lhs = x_T[kt][:, j * P:(j + 1) * P]
                        nc.tensor.matmul(
                            pv[:, 0:512], lhsT=lhs, rhs=w_bf[(2, kt)][:, 0:512],
                            start=(kt == 0), stop=(kt == DT - 1))
                        nc.tensor.matmul(
                            pv[:, 512:D], lhsT=lhs, rhs=w_bf[(2, kt)][:, 512:D],
                            start=(kt == 0), stop=(kt == DT - 1))
                    vdst = v_ext[j].rearrange("p (h c) -> p h c", c=HDE)[:, :, 0:HD]
                    vsrc = pv.rearrange("p (h c) -> p h c", c=HD)
                    bvv = bv_bc.rearrange("p (h c) -> p h c", c=HD)
                    nc.vector.tensor_add(vdst, vsrc, bvv)

            # ---- attention, pipelined over head pairs ----
            with (
                tc.tile_pool(name="qk_sb", bufs=2) as qkp,
                tc.tile_pool(name="p_sb", bufs=32) as pp,
                tc.tile_pool(name="ot_sb", bufs=3) as otp,
                tc.tile_pool(name="rc_sb", bufs=6) as rcp,
                tc.tile_pool(name="qk_ps", bufs=2, space="PSUM") as qkps,
                tc.tile_pool(name="s_ps", bufs=2, space="PSUM") as sps,
                tc.tile_pool(name="o_ps", bufs=1, space="PSUM") as ops,
            ):
                def qk_proj(t):
                    """q^T / k^T projection for d-tile t (bias fused, DVE drain)."""
                    qT = qkp.tile([P, N], BF16, tag="qT", name=f"qT_{t}")
                    kT = qkp.tile([P, N], BF16, tag="kT", name=f"kT_{t}")
                    for dst_sb, wi, b_sb in ((qT, 0, bq_sb), (kT, 1, bk_sb)):
                        for c in range(NC2):
                            qps = qkps.tile([P, 512], FP32, tag="qkps")
                            for kt in range(DT):
                                nc.tensor.matmul(
                                    qps,
                                    lhsT=w_bf[(wi, kt)][:, t * P:(t + 1) * P],
                                    rhs=x_T[kt][:, c * 512:(c + 1) * 512],
                                    start=(kt == 0), stop=(kt == DT - 1))
                            nc.vector.tensor_scalar_add(
                                dst_sb[:, c * 512:(c + 1) * 512], qps,
                                b_sb[:, t:t + 1])
                    return qT, kT

                def s_exp(t, qT, kT):
                    """S^T = k^T.T @ q^T (row-packed head pair) + exp."""
                    p_tiles = {}
                    for j in range(NT):
                        for c in range(NC2):
                            s_ps = sps.tile([P, 1024], FP32, tag="sps")
                            for h in range(2):
                                nc.tensor.matmul(
                                    s_ps[:, h * 512:(h + 1) * 512],
                                    lhsT=kT[h * HD:(h + 1) * HD, j * P:(j + 1) * P],
                                    rhs=qT[h * HD:(h + 1) * HD, c * 512:(c + 1) * 512],
                                    start=True, stop=True)
                            p_t = pp.tile([P, 1024], BF16, tag="p", name=f"p_{t}_{j}_{c}")
                            nc.scalar.activation(
                                out=p_t, in_=s_ps, func=AF.Exp,
                                bias=0.0, scale=0.125)
                            p_tiles[(j, c)] = p_t
                    return p_tiles

                # software pipeline: S/exp run one pair ahead of PV, so ACT
                # never waits for PV(t) to clear the PE queue.
                qT, kT = qk_proj(0)
                p_tiles = s_exp(0, qT, kT)
                for t in range(DT):
                    if t + 1 < DT:
                        next_qT, next_kT = qk_proj(t + 1)
                        next_p = s_exp(t + 1, next_qT, next_kT)
                    else:
                        next_p = None

                    # --- out^T_ext = [v | 1]^T @ P^T, one head at a time ---
                    for h in range(2):
                        gh = 2 * t + h
                        po = ops.tile([HDE, N], FP32, tag="ops", name=f"po_{t}_{h}")
                        for j in range(NT):
                            vl = v_ext[j][:, gh * HDE:(gh + 1) * HDE]
                            for c in range(NC2):
                                nc.tensor.matmul(
                                    po[:, c * 512:(c + 1) * 512],
                                    lhsT=vl,
                                    rhs=p_tiles[(j, c)][:, h * 512:(h + 1) * 512],
                                    start=(j == 0), stop=(j == NT - 1),
                                    skip_group_check=True)
                        ot = otp.tile([HDE, N], FP32, tag="ot", name=f"ot_{t}_{h}")
                        nc.vector.tensor_copy(out=ot, in_=po)
                        # transpose back to natural + normalize by denominators
                        for i in range(NT):
                            tps = qkps.tile([P, HDE], FP32, tag="qkps")
                            nc.tensor.transpose(
                                tps, ot[:, i * P:(i + 1) * P], ident[0:HDE, 0:HDE])
                            rc = rcp.tile([P, 1], FP32, tag="rc")
                            nc.vector.reciprocal(rc, tps[:, HD:HDE])
                            nc.vector.tensor_scalar_mul(
                                out_nat[i][:, gh * HD:(gh + 1) * HD],
                                tps[:, 0:HD], rc)
                            if t == DT - 1 and h == 1:
                                nc.gpsimd.dma_start(
                                    out=out_d[i * P:(i + 1) * P, :], in_=out_nat[i])

                    p_tiles = next_p

                    qT, kT = next_qT, next_kT

    if split_waits:
        _split_dma_waits(nc)
    return nc


_MAX_INLINE_WAITS = {"InstDMACopy": 1}
_DEFAULT_MAX_WAITS = 1


def _split_dma_waits(nc: bass.Bass) -> None:
    """walrus's instruction templates have limited semaphore-wait slots (one
    for the static-DMA pseudo, one for matmul's LDWEIGHTS, etc). Tile's sem
    assignment can attach more; hoist the excess into standalone
    InstEventSemaphore instructions on the issuing engine (sequencers execute
    in order, so the semantics are identical)."""
    for f in nc.m.functions:
        for bb in f.blocks:
            new_insts = []
            for inst in bb.instructions:
                si = getattr(inst, "sync_info", None)
                cap = _MAX_INLINE_WAITS.get(type(inst).__name__, _DEFAULT_MAX_WAITS)
                if si is not None and si.on_wait and len(si.on_wait) > cap:
                    waits = list(si.on_wait)
                    for wi, w in enumerate(waits[:-cap]):
                        ev = mybir.InstEventSemaphore(
                            name=f"{inst.name}-prewait{wi}")
                        ev.engine = inst.engine
                        ev.sync_info = mybir.SyncInfo(on_wait=[w], on_update=[])
                        new_insts.append(ev)
                    si.on_wait = waits[-cap:]
                    inst.sync_info = si
                new_insts.append(inst)
            bb.instructions[:] = new_insts


_BUILD_LOCK = threading.Lock()
_NC_CACHE: list = []


def _get_nc() -> bass.Bass:
    with _BUILD_LOCK:
        if not _NC_CACHE:
            _NC_CACHE.append(build_mha_bass())
    return _NC_CACHE[0]


def kernel(x, Wq, bq, Wk, bk, Wv, bv):
    x = np.ascontiguousarray(np.asarray(x, dtype=np.float32))
    Wq = np.ascontiguousarray(np.asarray(Wq, dtype=np.float32))
    Wk = np.ascontiguousarray(np.asarray(Wk, dtype=np.float32))
    Wv = np.ascontiguousarray(np.asarray(Wv, dtype=np.float32))
    bq = np.ascontiguousarray(np.asarray(bq, dtype=np.float32))
    bk = np.ascontiguousarray(np.asarray(bk, dtype=np.float32))
    bv = np.ascontiguousarray(np.asarray(bv, dtype=np.float32))
    assert x.shape == (B, N, D), x.shape

    nc = _get_nc()
    in_maps = [
        {"x": x[i], "Wq": Wq, "bq": bq, "Wk": Wk, "bk": bk, "Wv": Wv, "bv": bv}
        for i in range(B)
    ]
    res = run_bass_kernel_spmd(nc, in_maps, core_ids=list(range(B)))
    return np.stack([res.results[i]["out"] for i in range(B)], axis=0)


# revision 21
# speedup vs baseline: 1.0731x; 1.0062x over previous
"""Multi-head attention (B=8, N=1024, D=768, H=12) on 8 TRN2 NeuronCores.

Sharding: batch-parallel. Core i computes the full 12-head attention for
batch element i; weights are replicated. No collectives.

Per-core dataflow (all matmuls bf16 with fp32 PSUM accumulation):
  1. x [1024,768] f32 -> PE-transpose -> xT [768,1024] bf16 (k on partitions)
  2. v = x @ Wv + bv (natural layout [n, d], bias folded in during drain;
     P @ (v + 1 bv^T) / denom == P @ v / denom + bv exactly)
     qT = Wq^T x^T, kT = Wk^T x^T  ([d, n] layout, bias fused in DVE drain)
  3. per head pair t (heads 2t, 2t+1 live at partitions 0-63 / 64-127 of
     d-tile t):
       S^T[nk, nq] = kT^T @ qT   (K=64, two heads row-packed via base_partition)
       P^T = exp(S^T / 8)        (ACT, PSUM->SBUF bf16, scale fused)
       outU^T[d+1, nq] = [v | 1]^T @ P^T  (ones column yields softmax denoms)
       PE-transpose outU^T back to natural, multiply by 1/denom (per-partition
       scalar) into out_nat
     The qk projection of pair t+1 is emitted between S(t) and PV(t) so the
     in-order PE queue has independent work while ACT runs the exps.
  4. DMA out per n-tile.
"""

import threading

import numpy as np

import concourse.bass as bass
import concourse.tile as tile
from concourse import mybir
from concourse.bass_utils import run_bass_kernel_spmd
from concourse.masks import make_identity

B, N, D, H, HD = 8, 1024, 768, 12, 64
P = 128
NT = N // P          # 8  n-tiles
DT = D // P          # 6  d-tiles == head pairs
NC2 = N // 512       # 2  512-wide n chunks
HDE = HD + 1         # 65 head cols incl. ones column
FP32 = mybir.dt.float32
BF16 = mybir.dt.bfloat16
AF = mybir.ActivationFunctionType


def build_mha_bass(split_waits: bool = True) -> bass.Bass:
    nc = bass.Bass()

    x_d = nc.declare_dram_parameter("x", [N, D], FP32, isOutput=False)
    wq_d = nc.declare_dram_parameter("Wq", [D, D], FP32, isOutput=False)
    bq_d = nc.declare_dram_parameter("bq", [D], FP32, isOutput=False)
    wk_d = nc.declare_dram_parameter("Wk", [D, D], FP32, isOutput=False)
    bk_d = nc.declare_dram_parameter("bk", [D], FP32, isOutput=False)
    wv_d = nc.declare_dram_parameter("Wv", [D, D], FP32, isOutput=False)
    bv_d = nc.declare_dram_parameter("bv", [D], FP32, isOutput=False)
    out_d = nc.declare_dram_parameter("out", [N, D], FP32, isOutput=True)

    with tile.TileContext(nc) as tc:
        with tc.tile_pool(name="singles", bufs=1) as singles:
            ident = singles.tile([P, P], FP32)
            make_identity(nc, ident)

            # biases: bq/bk as [128, DT] (per-partition scalars per d-tile),
            # bv broadcast across partitions (folded into the v drain).
            bq_sb = singles.tile([P, DT], FP32)
            bk_sb = singles.tile([P, DT], FP32)
            bv_bc = singles.tile([P, D], FP32)

            w_bf = {}
            for wi in (2, 0, 1):
                for kt in range(DT):
                    w_bf[(wi, kt)] = singles.tile([P, D], BF16, name=f"wbf{wi}_{kt}")
            x_T = [singles.tile([P, N], BF16, name=f"xT_{c}") for c in range(DT)]
            v_ext = [singles.tile([P, H * HDE], BF16, name=f"vext_{j}") for j in range(NT)]
            for j in range(NT):
                ones_cols = v_ext[j].rearrange("p (h c) -> p h c", c=HDE)[:, :, HD:HDE]
                nc.vector.memset(ones_cols, 1.0)

            out_nat = [singles.tile([P, D], FP32, name=f"onat_{i}") for i in range(NT)]

            with (
                tc.tile_pool(name="xstage", bufs=NT) as xstage,
                tc.tile_pool(name="wstage", bufs=18) as wstage,
                tc.tile_pool(name="xt_ps", bufs=4, space="PSUM") as xtps,
            ):
                # ---- input DMAs, latency-critical first: x, then Wv, Wq, Wk ----
                x_st = []
                for i in range(NT):
                    xst = xstage.tile([P, D], FP32, tag="xs", name=f"xst_{i}")
                    nc.gpsimd.dma_start(out=xst, in_=x_d[i * P:(i + 1) * P, :])
                    x_st.append(xst)

                w_st = {}
                for wi, wd in ((2, wv_d), (0, wq_d), (1, wk_d)):
                    for kt in range(DT):
                        wst = wstage.tile([P, D], FP32, tag="wst", name=f"wst{wi}_{kt}")
                        nc.gpsimd.dma_start(out=wst, in_=wd[kt * P:(kt + 1) * P, :])
                        w_st[(wi, kt)] = wst

                nc.gpsimd.dma_start(out=bq_sb, in_=bq_d[:].rearrange("(t p) -> p t", p=P))
                nc.gpsimd.dma_start(out=bk_sb, in_=bk_d[:].rearrange("(t p) -> p t", p=P))
                bv_ap = bv_d[:]
                nc.gpsimd.dma_start(
                    out=bv_bc,
                    in_=bass.AP(tensor=bv_ap.tensor, offset=bv_ap.offset, ap=[[0, P], [1, D]]),
                )

                # ---- weights -> bf16 (Wv first; DVE) ----
                for wi in (2, 0, 1):
                    for kt in range(DT):
                        nc.vector.tensor_copy(out=w_bf[(wi, kt)], in_=w_st[(wi, kt)])

                # ---- x -> xT (bf16) via PE transpose ----
                for i in range(NT):
                    for c in range(DT):
                        xps = xtps.tile([P, P], FP32, tag="xtps")
                        nc.tensor.transpose(xps, x_st[i][:, c * P:(c + 1) * P], ident)
                        nc.vector.tensor_copy(out=x_T[c][:, i * P:(i + 1) * P], in_=xps)

            # ---- attention, pipelined over head pairs ----
            with (
                tc.tile_pool(name="qk_sb", bufs=2) as qkp,
                tc.tile_pool(name="p_sb", bufs=32) as pp,
                tc.tile_pool(name="ot_sb", bufs=3) as otp,
                tc.tile_pool(name="rc_sb", bufs=6) as rcp,
                tc.tile_pool(name="qk_ps", bufs=2, space="PSUM") as qkps,
                tc.tile_pool(name="s_ps", bufs=2, space="PSUM") as sps,
                tc.tile_pool(name="o_ps", bufs=1, space="PSUM") as ops,
            ):
                def qk_proj(t):
                    """q^T / k^T projection for d-tile t (bias fused, DVE drain)."""
                    qT = qkp.tile([P, N], BF16, tag="qT", name=f"qT_{t}")
                    kT = qkp.tile([P, N], BF16, tag="kT", name=f"kT_{t}")
                    for dst_sb, wi, b_sb in ((qT, 0, bq_sb), (kT, 1, bk_sb)):
                        for c in range(NC2):
                            qps = qkps.tile([P, 512], FP32, tag="qkps")
                            for kt in range(DT):
                                nc.tensor.matmul(
                                    qps,
                                    lhsT=w_bf[(wi, kt)][:, t * P:(t + 1) * P],
                                    rhs=x_T[kt][:, c * 512:(c + 1) * 512],
                                    start=(kt == 0), stop=(kt == DT - 1))
                            nc.vector.tensor_scalar_add(
                                dst_sb[:, c * 512:(c + 1) * 512], qps,
                                b_sb[:, t:t + 1])
                    return qT, kT

                def s_exp(t, qT, kT):
                    """S^T = k^T.T @ q^T (row-packed head pair) + exp."""
                    p_tiles = {}
                    for j in range(NT):
                        for c in range(NC2):
                            s_ps = sps.tile([P, 1024], FP32, tag="sps")
                            for h in range(2):
                                nc.tensor.matmul(
                                    s_ps[:, h * 512:(h + 1) * 512],
                                    lhsT=kT[h * HD:(h + 1) * HD, j * P:(j + 1) * P],
                                    rhs=qT[h * HD:(h + 1) * HD, c * 512:(c + 1) * 512],
                                    start=True, stop=True)
                            p_t = pp.tile([P, 1024], BF16, tag="p", name=f"p_{t}_{j}_{c}")
                            nc.scalar.activation(
                                out=p_t, in_=s_ps, func=AF.Exp,
                                bias=0.0, scale=0.125)
                            p_tiles[(j, c)] = p_t
                    return p_tiles

                # software pipeline: S/exp run one pair ahead of PV, so ACT
                # never waits for PV(t) to clear the PE queue. qk(0)+S(0) go
                # FIRST so the exps start as early as possible; v (only needed
                # by PV(0)) is computed while ACT chews on pair 0, sharing the
                # o_ps PSUM banks.
                qT, kT = qk_proj(0)
                p_tiles = s_exp(0, qT, kT)

                for j in range(NT):
                    pv = ops.tile([P, D], FP32, tag="ops", name=f"pv_{j}")
                    for kt in range(DT):
                        lhs = x_T[kt][:, j * P:(j + 1) * P]
                        nc.tensor.matmul(
                            pv[:, 0:512], lhsT=lhs, rhs=w_bf[(2, kt)][:, 0:512],
                            start=(kt == 0), stop=(kt == DT - 1))
                        nc.tensor.matmul(
                            pv[:, 512:D], lhsT=lhs, rhs=w_bf[(2, kt)][:, 512:D],
                            start=(kt == 0), stop=(kt == DT - 1))
                    vdst = v_ext[j].rearrange("p (h c) -> p h c", c=HDE)[:, :, 0:HD]
                    vsrc = pv.rearrange("p (h c) -> p h c", c=HD)
                    bvv = bv_bc.rearrange("p (h c) -> p h c", c=HD)
                    nc.vector.tensor_add(vdst, vsrc, bvv)

                for t in range(DT):
                    if t + 1 < DT:
                        next_qT, next_kT = qk_proj(t + 1)
                        next_p = s_exp(t + 1, next_qT, next_kT)
                    else:
                        next_p = None

                    # --- out^T_ext = [v | 1]^T @ P^T, one head at a time ---
                    for h in range(2):
                        gh = 2 * t + h
                        po = ops.tile([HDE, N], FP32, tag="ops", name=f"po_{t}_{h}")
                        for j in range(NT):
                            vl = v_ext[j][:, gh * HDE:(gh + 1) * HDE]
                            for c in range(NC2):
                                nc.tensor.matmul(
                                    po[:, c * 512:(c + 1) * 512],
                                    lhsT=vl,
                                    rhs=p_tiles[(j, c)][:, h * 512:(h + 1) * 512],
                                    start=(j == 0), stop=(j == NT - 1),
                                    skip_group_check=True)
                        ot = otp.tile([HDE, N], FP32, tag="ot", name=f"ot_{t}_{h}")
                        nc.vector.tensor_copy(out=ot, in_=po)
                        # transpose back to natural + normalize by denominators
                        for i in range(NT):
                            tps = qkps.tile([P, HDE], FP32, tag="qkps")
                            nc.tensor.transpose(
                                tps, ot[:, i * P:(i + 1) * P], ident[0:HDE, 0:HDE])
                            rc = rcp.tile([P, 1], FP32, tag="rc")
                            nc.vector.reciprocal(rc, tps[:, HD:HDE])
                            nc.vector.tensor_scalar_mul(
                                out_nat[i][:, gh * HD:(gh + 1) * HD],
                                tps[:, 0:HD], rc)
                            if t == DT - 1 and h == 1:
                                nc.gpsimd.dma_start(
                                    out=out_d[i * P:(i + 1) * P, :], in_=out_nat[i])

                    p_tiles = next_p

                    qT, kT = next_qT, next_kT

    if split_waits:
        _split_dma_waits(nc)
    return nc


_MAX_INLINE_WAITS = {"InstDMACopy": 1}
_DEFAULT_MAX_WAITS = 1


def _split_dma_waits(nc: bass.Bass) -> None:
    """walrus's instruction templates have limited semaphore-wait slots (one
    for the static-DMA pseudo, one for matmul's LDWEIGHTS, etc). Tile's sem
    assignment can attach more; hoist the excess into standalone
    InstEventSemaphore instructions on the issuing engine (sequencers execute
    in order, so the semantics are identical)."""
    for f in nc.m.functions:
        for bb in f.blocks:
            new_insts = []
            for inst in bb.instructions:
                si = getattr(inst, "sync_info", None)
                cap = _MAX_INLINE_WAITS.get(type(inst).__name__, _DEFAULT_MAX_WAITS)
                if si is not None and si.on_wait and len(si.on_wait) > cap:
                    waits = list(si.on_wait)
                    for wi, w in enumerate(waits[:-cap]):
                        ev = mybir.InstEventSemaphore(
                            name=f"{inst.name}-prewait{wi}")
                        ev.engine = inst.engine
                        ev.sync_info = mybir.SyncInfo(on_wait=[w], on_update=[])
                        new_insts.append(ev)
                    si.on_wait = waits[-cap:]
                    inst.sync_info = si
                new_insts.append(inst)
            bb.instructions[:] = new_insts


_BUILD_LOCK = threading.Lock()
_NC_CACHE: list = []


def _get_nc() -> bass.Bass:
    with _BUILD_LOCK:
        if not _NC_CACHE:
            _NC_CACHE.append(build_mha_bass())
    return _NC_CACHE[0]


def kernel(x, Wq, bq, Wk, bk, Wv, bv):
    x = np.ascontiguousarray(np.asarray(x, dtype=np.float32))
    Wq = np.ascontiguousarray(np.asarray(Wq, dtype=np.float32))
    Wk = np.ascontiguousarray(np.asarray(Wk, dtype=np.float32))
    Wv = np.ascontiguousarray(np.asarray(Wv, dtype=np.float32))
    bq = np.ascontiguousarray(np.asarray(bq, dtype=np.float32))
    bk = np.ascontiguousarray(np.asarray(bk, dtype=np.float32))
    bv = np.ascontiguousarray(np.asarray(bv, dtype=np.float32))
    assert x.shape == (B, N, D), x.shape

    nc = _get_nc()
    in_maps = [
        {"x": x[i], "Wq": Wq, "bq": bq, "Wk": Wk, "bk": bk, "Wv": Wv, "bv": bv}
        for i in range(B)
    ]
    res = run_bass_kernel_spmd(nc, in_maps, core_ids=list(range(B)))
    return np.stack([res.results[i]["out"] for i in range(B)], axis=0)


# revision 22
# speedup vs baseline: 1.0948x; 1.0202x over previous
"""Multi-head attention (B=8, N=1024, D=768, H=12) on 8 TRN2 NeuronCores.

Sharding: batch-parallel. Core i computes the full 12-head attention for
batch element i; weights are replicated. No collectives.

Per-core dataflow (all matmuls bf16 with fp32 PSUM accumulation):
  1. x [1024,768] f32 -> PE-transpose -> xT [768,1024] bf16 (k on partitions)
  2. v = x @ Wv + bv (natural layout [n, d], bias folded in during drain;
     P @ (v + 1 bv^T) / denom == P @ v / denom + bv exactly)
     qT = Wq^T x^T, kT = Wk^T x^T  ([d, n] layout, bias fused in DVE drain)
  3. per head pair t (heads 2t, 2t+1 live at partitions 0-63 / 64-127 of
     d-tile t):
       S^T[nk, nq] = kT^T @ qT   (K=64, two heads row-packed via base_partition)
       P^T = exp(S^T / 8)        (ACT, PSUM->SBUF bf16, scale fused)
       outU^T[d+1, nq] = [v | 1]^T @ P^T  (ones column yields softmax denoms)
       PE-transpose outU^T back to natural, multiply by 1/denom (per-partition
       scalar) into out_nat
     The qk projection of pair t+1 is emitted between S(t) and PV(t) so the
     in-order PE queue has independent work while ACT runs the exps.
  4. DMA out per n-tile.
"""

import threading

import numpy as np

import concourse.bass as bass
import concourse.tile as tile
from concourse import mybir
from concourse.bass_utils import run_bass_kernel_spmd
from concourse.masks import make_identity

B, N, D, H, HD = 8, 1024, 768, 12, 64
P = 128
NT = N // P          # 8  n-tiles
DT = D // P          # 6  d-tiles == head pairs
NC2 = N // 512       # 2  512-wide n chunks
HDE = HD + 1         # 65 head cols incl. ones column
FP32 = mybir.dt.float32
BF16 = mybir.dt.bfloat16
AF = mybir.ActivationFunctionType


def build_mha_bass(split_waits: bool = True) -> bass.Bass:
    nc = bass.Bass()

    x_d = nc.declare_dram_parameter("x", [N, D], FP32, isOutput=False)
    wq_d = nc.declare_dram_parameter("Wq", [D, D], FP32, isOutput=False)
    bq_d = nc.declare_dram_parameter("bq", [D], FP32, isOutput=False)
    wk_d = nc.declare_dram_parameter("Wk", [D, D], FP32, isOutput=False)
    bk_d = nc.declare_dram_parameter("bk", [D], FP32, isOutput=False)
    wv_d = nc.declare_dram_parameter("Wv", [D, D], FP32, isOutput=False)
    bv_d = nc.declare_dram_parameter("bv", [D], FP32, isOutput=False)
    out_d = nc.declare_dram_parameter("out", [N, D], FP32, isOutput=True)

    with tile.TileContext(nc) as tc:
        with tc.tile_pool(name="singles", bufs=1) as singles:
            ident = singles.tile([P, P], FP32)
            make_identity(nc, ident)

            # biases: bq/bk as [128, DT] (per-partition scalars per d-tile),
            # bv broadcast across partitions (folded into the v drain).
            bq_sb = singles.tile([P, DT], FP32)
            bk_sb = singles.tile([P, DT], FP32)
            bv_bc = singles.tile([P, D], FP32)

            w_bf = {}
            for wi in (2, 0, 1):
                for kt in range(DT):
                    w_bf[(wi, kt)] = singles.tile([P, D], BF16, name=f"wbf{wi}_{kt}")
            x_T = [singles.tile([P, N], BF16, name=f"xT_{c}") for c in range(DT)]
            v_ext = [singles.tile([P, H * HDE], BF16, name=f"vext_{j}") for j in range(NT)]
            for j in range(NT):
                ones_cols = v_ext[j].rearrange("p (h c) -> p h c", c=HDE)[:, :, HD:HDE]
                nc.vector.memset(ones_cols, 1.0)

            out_nat = [singles.tile([P, D], FP32, name=f"onat_{i}") for i in range(NT)]

            with (
                tc.tile_pool(name="xstage", bufs=NT) as xstage,
                tc.tile_pool(name="wstage", bufs=18) as wstage,
                tc.tile_pool(name="xt_ps", bufs=4, space="PSUM") as xtps,
            ):
                # ---- input DMAs, latency-critical first: x, then Wv, Wq, Wk ----
                x_st = []
                for i in range(NT):
                    xst = xstage.tile([P, D], FP32, tag="xs", name=f"xst_{i}")
                    nc.gpsimd.dma_start(out=xst, in_=x_d[i * P:(i + 1) * P, :])
                    x_st.append(xst)

                w_st = {}
                for wi, wd in ((2, wv_d), (0, wq_d), (1, wk_d)):
                    for kt in range(DT):
                        wst = wstage.tile([P, D], FP32, tag="wst", name=f"wst{wi}_{kt}")
                        nc.gpsimd.dma_start(out=wst, in_=wd[kt * P:(kt + 1) * P, :])
                        w_st[(wi, kt)] = wst

                nc.gpsimd.dma_start(out=bq_sb, in_=bq_d[:].rearrange("(t p) -> p t", p=P))
                nc.gpsimd.dma_start(out=bk_sb, in_=bk_d[:].rearrange("(t p) -> p t", p=P))
                bv_ap = bv_d[:]
                nc.gpsimd.dma_start(
                    out=bv_bc,
                    in_=bass.AP(tensor=bv_ap.tensor, offset=bv_ap.offset, ap=[[0, P], [1, D]]),
                )

                # ---- weights -> bf16 (Wv first; DVE) ----
                for wi in (2, 0, 1):
                    for kt in range(DT):
                        nc.vector.tensor_copy(out=w_bf[(wi, kt)], in_=w_st[(wi, kt)])

                # ---- x -> xT (bf16) via PE transpose ----
                for i in range(NT):
                    for c in range(DT):
                        xps = xtps.tile([P, P], FP32, tag="xtps")
                        nc.tensor.transpose(xps, x_st[i][:, c * P:(c + 1) * P], ident)
                        nc.vector.tensor_copy(out=x_T[c][:, i * P:(i + 1) * P], in_=xps)

            # ---- attention, pipelined over head pairs ----
            with (
                tc.tile_pool(name="qk_sb", bufs=2) as qkp,
                tc.tile_pool(name="p_sb", bufs=32) as pp,
                tc.tile_pool(name="ot_sb", bufs=3) as otp,
                tc.tile_pool(name="rc_sb", bufs=6) as rcp,
                tc.tile_pool(name="qk_ps", bufs=2, space="PSUM") as qkps,
                tc.tile_pool(name="s_ps", bufs=2, space="PSUM") as sps,
                tc.tile_pool(name="o_ps", bufs=1, space="PSUM") as ops,
            ):
                def qk_proj(t):
                    """q^T / k^T projection for d-tile t (bias fused, DVE drain)."""
                    qT = qkp.tile([P, N], BF16, tag="qT", name=f"qT_{t}")
                    kT = qkp.tile([P, N], BF16, tag="kT", name=f"kT_{t}")
                    for dst_sb, wi, b_sb in ((qT, 0, bq_sb), (kT, 1, bk_sb)):
                        for c in range(NC2):
                            qps = qkps.tile([P, 512], FP32, tag="qkps")
                            for kt in range(DT):
                                nc.tensor.matmul(
                                    qps,
                                    lhsT=w_bf[(wi, kt)][:, t * P:(t + 1) * P],
                                    rhs=x_T[kt][:, c * 512:(c + 1) * 512],
                                    start=(kt == 0), stop=(kt == DT - 1))
                            nc.vector.tensor_scalar_add(
                                dst_sb[:, c * 512:(c + 1) * 512], qps,
                                b_sb[:, t:t + 1])
                    return qT, kT

                def s_exp(t, qT, kT):
                    """S^T = k^T.T @ q^T (row-packed head pair) + exp."""
                    p_tiles = {}
                    for j in range(NT):
                        for c in range(NC2):
                            s_ps = sps.tile([P, 1024], FP32, tag="sps")
                            for h in range(2):
                                nc.tensor.matmul(
                                    s_ps[:, h * 512:(h + 1) * 512],
                                    lhsT=kT[h * HD:(h + 1) * HD, j * P:(j + 1) * P],
                                    rhs=qT[h * HD:(h + 1) * HD, c * 512:(c + 1) * 512],
                                    start=True, stop=True)
                            p_t = pp.tile([P, 1024], BF16, tag="p", name=f"p_{t}_{j}_{c}")
                            nc.scalar.activation(
                                out=p_t, in_=s_ps, func=AF.Exp,
                                bias=0.0, scale=0.125)
                            p_tiles[(j, c)] = p_t
                    return p_tiles

                # software pipeline: S/exp run one pair ahead of PV, so ACT
                # never waits for PV(t) to clear the PE queue. qk(0)+S(0) go
                # FIRST so the exps start as early as possible; v (only needed
                # by PV(0)) is computed while ACT chews on pair 0, sharing the
                # o_ps PSUM banks.
                qT, kT = qk_proj(0)
                p_tiles = s_exp(0, qT, kT)

                for j in range(NT):
                    pv = ops.tile([P, D], FP32, tag="ops", name=f"pv_{j}")
                    for kt in range(DT):
                        lhs = x_T[kt][:, j * P:(j + 1) * P]
                        nc.tensor.matmul(
                            pv[:, 0:512], lhsT=lhs, rhs=w_bf[(2, kt)][:, 0:512],
                            start=(kt == 0), stop=(kt == DT - 1))
                        nc.tensor.matmul(
                            pv[:, 512:D], lhsT=lhs, rhs=w_bf[(2, kt)][:, 512:D],
                            start=(kt == 0), stop=(kt == DT - 1))
                    vdst = v_ext[j].rearrange("p (h c) -> p h c", c=HDE)[:, :, 0:HD]
                    vsrc = pv.rearrange("p (h c) -> p h c", c=HD)
                    bvv = bv_bc.rearrange("p (h c) -> p h c", c=HD)
                    nc.vector.tensor_add(vdst, vsrc, bvv)

                for t in range(DT):
                    if t + 1 < DT:
                        next_qT, next_kT = qk_proj(t + 1)
                        next_p = s_exp(t + 1, next_qT, next_kT)
                    else:
                        next_p = None

                    # --- out^T_ext = [v | 1]^T @ P^T, one head at a time ---
                    for h in range(2):
                        gh = 2 * t + h
                        po = ops.tile([HDE, N], FP32, tag="ops", name=f"po_{t}_{h}")
                        for j in range(NT):
                            vl = v_ext[j][:, gh * HDE:(gh + 1) * HDE]
                            for c in range(NC2):
                                nc.tensor.matmul(
                                    po[:, c * 512:(c + 1) * 512],
                                    lhsT=vl,
                                    rhs=p_tiles[(j, c)][:, h * 512:(h + 1) * 512],
                                    start=(j == 0), stop=(j == NT - 1),
                                    skip_group_check=True)
                        ot = otp.tile([HDE, N], FP32, tag="ot", name=f"ot_{t}_{h}")
                        nc.vector.tensor_copy(out=ot, in_=po)
                        # transpose back to natural + normalize by denominators
                        for i in range(NT):
                            tps = qkps.tile([P, HDE], FP32, tag="qkps")
                            nc.tensor.transpose(
                                tps, ot[:, i * P:(i + 1) * P], ident[0:HDE, 0:HDE])
                            rc = rcp.tile([P, 1], FP32, tag="rc")
                            nc.vector.reciprocal(rc, tps[:, HD:HDE])
                            nc.vector.tensor_scalar_mul(
                                out_nat[i][:, gh * HD:(gh + 1) * HD],
                                tps[:, 0:HD], rc)
                            if h == 1:
                                # stream this pair's 128-column slice out now,
                                # so the kernel tail only waits on pair 5's
                                # slices instead of whole 384KB rows.
                                nc.gpsimd.dma_start(
                                    out=out_d[i * P:(i + 1) * P, t * P:(t + 1) * P],
                                    in_=out_nat[i][:, t * P:(t + 1) * P])

                    p_tiles = next_p

                    qT, kT = next_qT, next_kT

    if split_waits:
        _split_dma_waits(nc)
    return nc


_MAX_INLINE_WAITS = {"InstDMACopy": 1}
_DEFAULT_MAX_WAITS = 1


def _split_dma_waits(nc: bass.Bass) -> None:
    """walrus's instruction templates have limited semaphore-wait slots (one
    for the static-DMA pseudo, one for matmul's LDWEIGHTS, etc). Tile's sem
    assignment can attach more; hoist the excess into standalone
    InstEventSemaphore instructions on the issuing engine (sequencers execute
    in order, so the semantics are identical)."""
    for f in nc.m.functions:
        for bb in f.blocks:
            new_insts = []
            for inst in bb.instructions:
                si = getattr(inst, "sync_info", None)
                cap = _MAX_INLINE_WAITS.get(type(inst).__name__, _DEFAULT_MAX_WAITS)
                if si is not None and si.on_wait and len(si.on_wait) > cap:
                    waits = list(si.on_wait)
                    for wi, w in enumerate(waits[:-cap]):
                        ev = mybir.InstEventSemaphore(
                            name=f"{inst.name}-prewait{wi}")
                        ev.engine = inst.engine
                        ev.sync_info = mybir.SyncInfo(on_wait=[w], on_update=[])
                        new_insts.append(ev)
                    si.on_wait = waits[-cap:]
                    inst.sync_info = si
                new_insts.append(inst)
            bb.instructions[:] = new_insts


_BUILD_LOCK = threading.Lock()
_NC_CACHE: list = []


def _get_nc() -> bass.Bass:
    with _BUILD_LOCK:
        if not _NC_CACHE:
            _NC_CACHE.append(build_mha_bass())
    return _NC_CACHE[0]


def kernel(x, Wq, bq, Wk, bk, Wv, bv):
    x = np.ascontiguousarray(np.asarray(x, dtype=np.float32))
    Wq = np.ascontiguousarray(np.asarray(Wq, dtype=np.float32))
    Wk = np.ascontiguousarray(np.asarray(Wk, dtype=np.float32))
    Wv = np.ascontiguousarray(np.asarray(Wv, dtype=np.float32))
    bq = np.ascontiguousarray(np.asarray(bq, dtype=np.float32))
    bk = np.ascontiguousarray(np.asarray(bk, dtype=np.float32))
    bv = np.ascontiguousarray(np.asarray(bv, dtype=np.float32))
    assert x.shape == (B, N, D), x.shape

    nc = _get_nc()
    in_maps = [
        {"x": x[i], "Wq": Wq, "bq": bq, "Wk": Wk, "bk": bk, "Wv": Wv, "bv": bv}
        for i in range(B)
    ]
    res = run_bass_kernel_spmd(nc, in_maps, core_ids=list(range(B)))
    return np.stack([res.results[i]["out"] for i in range(B)], axis=0)


# revision 23
# speedup vs baseline: 1.1002x; 1.0049x over previous
"""Multi-head attention (B=8, N=1024, D=768, H=12) on 8 TRN2 NeuronCores.

Sharding: batch-parallel. Core i computes the full 12-head attention for
batch element i; weights are replicated. No collectives.

Per-core dataflow (all matmuls bf16 with fp32 PSUM accumulation):
  1. x [1024,768] f32 -> PE-transpose -> xT [768,1024] bf16 (k on partitions)
  2. v = x @ Wv + bv (natural layout [n, d], bias folded in during drain;
     P @ (v + 1 bv^T) / denom == P @ v / denom + bv exactly)
     qT = Wq^T x^T, kT = Wk^T x^T  ([d, n] layout, bias fused in DVE drain)
  3. per head pair t (heads 2t, 2t+1 live at partitions 0-63 / 64-127 of
     d-tile t):
       S^T[nk, nq] = kT^T @ qT   (K=64, two heads row-packed via base_partition)
       P^T = exp(S^T / 8)        (ACT, PSUM->SBUF bf16, scale fused)
       outU^T[d+1, nq] = [v | 1]^T @ P^T  (ones column yields softmax denoms)
       PE-transpose outU^T back to natural, multiply by 1/denom (per-partition
       scalar) into out_nat
     The qk projection of pair t+1 is emitted between S(t) and PV(t) so the
     in-order PE queue has independent work while ACT runs the exps.
  4. DMA out per n-tile.
"""

import threading

import numpy as np

import concourse.bass as bass
import concourse.tile as tile
from concourse import mybir
from concourse.bass_utils import run_bass_kernel_spmd
from concourse.masks import make_identity

B, N, D, H, HD = 8, 1024, 768, 12, 64
P = 128
NT = N // P          # 8  n-tiles
DT = D // P          # 6  d-tiles == head pairs
NC2 = N // 512       # 2  512-wide n chunks
HDE = HD + 1         # 65 head cols incl. ones column
FP32 = mybir.dt.float32
BF16 = mybir.dt.bfloat16
AF = mybir.ActivationFunctionType


def build_mha_bass(split_waits: bool = True) -> bass.Bass:
    nc = bass.Bass()

    x_d = nc.declare_dram_parameter("x", [N, D], FP32, isOutput=False)
    wq_d = nc.declare_dram_parameter("Wq", [D, D], FP32, isOutput=False)
    bq_d = nc.declare_dram_parameter("bq", [D], FP32, isOutput=False)
    wk_d = nc.declare_dram_parameter("Wk", [D, D], FP32, isOutput=False)
    bk_d = nc.declare_dram_parameter("bk", [D], FP32, isOutput=False)
    wv_d = nc.declare_dram_parameter("Wv", [D, D], FP32, isOutput=False)
    bv_d = nc.declare_dram_parameter("bv", [D], FP32, isOutput=False)
    out_d = nc.declare_dram_parameter("out", [N, D], FP32, isOutput=True)

    with tile.TileContext(nc) as tc:
        with tc.tile_pool(name="singles", bufs=1) as singles:
            ident = singles.tile([P, P], FP32)
            make_identity(nc, ident)

            # biases: bq/bk as [128, DT] (per-partition scalars per d-tile),
            # bv broadcast across partitions (folded into the v drain).
            bq_sb = singles.tile([P, DT], FP32)
            bk_sb = singles.tile([P, DT], FP32)
            bv_bc = singles.tile([P, D], FP32)

            w_bf = {}
            for wi in (2, 0, 1):
                for kt in range(DT):
                    w_bf[(wi, kt)] = singles.tile([P, D], BF16, name=f"wbf{wi}_{kt}")
            x_T = [singles.tile([P, N], BF16, name=f"xT_{c}") for c in range(DT)]
            v_ext = [singles.tile([P, H * HDE], BF16, name=f"vext_{j}") for j in range(NT)]
            for j in range(NT):
                ones_cols = v_ext[j].rearrange("p (h c) -> p h c", c=HDE)[:, :, HD:HDE]
                nc.vector.memset(ones_cols, 1.0)

            out_nat = [singles.tile([P, D], FP32, name=f"onat_{i}") for i in range(NT)]

            with (
                tc.tile_pool(name="xstage", bufs=NT) as xstage,
                tc.tile_pool(name="wstage", bufs=18) as wstage,
                tc.tile_pool(name="xt_ps", bufs=4, space="PSUM") as xtps,
            ):
                # ---- input DMAs, latency-critical first: x, then Wv, Wq, Wk ----
                x_st = []
                for i in range(NT):
                    xst = xstage.tile([P, D], FP32, tag="xs", name=f"xst_{i}")
                    nc.gpsimd.dma_start(out=xst, in_=x_d[i * P:(i + 1) * P, :])
                    x_st.append(xst)

                w_st = {}
                for wi, wd in ((2, wv_d), (0, wq_d), (1, wk_d)):
                    for kt in range(DT):
                        wst = wstage.tile([P, D], FP32, tag="wst", name=f"wst{wi}_{kt}")
                        nc.gpsimd.dma_start(out=wst, in_=wd[kt * P:(kt + 1) * P, :])
                        w_st[(wi, kt)] = wst

                nc.gpsimd.dma_start(out=bq_sb, in_=bq_d[:].rearrange("(t p) -> p t", p=P))
                nc.gpsimd.dma_start(out=bk_sb, in_=bk_d[:].rearrange("(t p) -> p t", p=P))
                bv_ap = bv_d[:]
                nc.gpsimd.dma_start(
                    out=bv_bc,
                    in_=bass.AP(tensor=bv_ap.tensor, offset=bv_ap.offset, ap=[[0, P], [1, D]]),
                )

                # ---- weights -> bf16 (Wv first; DVE) ----
                for wi in (2, 0, 1):
                    for kt in range(DT):
                        nc.vector.tensor_copy(out=w_bf[(wi, kt)], in_=w_st[(wi, kt)])

                # ---- x -> xT (bf16) via PE transpose ----
                for i in range(NT):
                    for c in range(DT):
                        xps = xtps.tile([P, P], FP32, tag="xtps")
                        nc.tensor.transpose(xps, x_st[i][:, c * P:(c + 1) * P], ident)
                        nc.vector.tensor_copy(out=x_T[c][:, i * P:(i + 1) * P], in_=xps)

            # ---- attention, pipelined over head pairs ----
            with (
                tc.tile_pool(name="qk_sb", bufs=2) as qkp,
                tc.tile_pool(name="p_sb", bufs=32) as pp,
                tc.tile_pool(name="ot_sb", bufs=3) as otp,
                tc.tile_pool(name="rc_sb", bufs=6) as rcp,
                # s_ps first so it takes the banks aliased with the staging
                # xt_ps pool; qk_ps/o_ps then land on fresh banks and qk(0)
                # isn't WAR-gated on the last x-transposes.
                tc.tile_pool(name="s_ps", bufs=2, space="PSUM") as sps,
                tc.tile_pool(name="qk_ps", bufs=2, space="PSUM") as qkps,
                tc.tile_pool(name="o_ps", bufs=1, space="PSUM") as ops,
            ):
                def qk_proj(t):
                    """q^T / k^T projection for d-tile t (bias fused, DVE drain)."""
                    qT = qkp.tile([P, N], BF16, tag="qT", name=f"qT_{t}")
                    kT = qkp.tile([P, N], BF16, tag="kT", name=f"kT_{t}")
                    for dst_sb, wi, b_sb in ((qT, 0, bq_sb), (kT, 1, bk_sb)):
                        for c in range(NC2):
                            qps = qkps.tile([P, 512], FP32, tag="qkps")
                            for kt in range(DT):
                                nc.tensor.matmul(
                                    qps,
                                    lhsT=w_bf[(wi, kt)][:, t * P:(t + 1) * P],
                                    rhs=x_T[kt][:, c * 512:(c + 1) * 512],
                                    start=(kt == 0), stop=(kt == DT - 1))
                            nc.vector.tensor_scalar_add(
                                dst_sb[:, c * 512:(c + 1) * 512], qps,
                                b_sb[:, t:t + 1])
                    return qT, kT

                def s_exp(t, qT, kT):
                    """S^T = k^T.T @ q^T (row-packed head pair) + exp."""
                    p_tiles = {}
                    for j in range(NT):
                        for c in range(NC2):
                            s_ps = sps.tile([P, 1024], FP32, tag="sps")
                            for h in range(2):
                                nc.tensor.matmul(
                                    s_ps[:, h * 512:(h + 1) * 512],
                                    lhsT=kT[h * HD:(h + 1) * HD, j * P:(j + 1) * P],
                                    rhs=qT[h * HD:(h + 1) * HD, c * 512:(c + 1) * 512],
                                    start=True, stop=True)
                            p_t = pp.tile([P, 1024], BF16, tag="p", name=f"p_{t}_{j}_{c}")
                            nc.scalar.activation(
                                out=p_t, in_=s_ps, func=AF.Exp,
                                bias=0.0, scale=0.125)
                            p_tiles[(j, c)] = p_t
                    return p_tiles

                # software pipeline: S/exp run one pair ahead of PV, so ACT
                # never waits for PV(t) to clear the PE queue. qk(0)+S(0) go
                # FIRST so the exps start as early as possible; v (only needed
                # by PV(0)) is computed while ACT chews on pair 0, sharing the
                # o_ps PSUM banks.
                qT, kT = qk_proj(0)
                p_tiles = s_exp(0, qT, kT)

                for j in range(NT):
                    pv = ops.tile([P, D], FP32, tag="ops", name=f"pv_{j}")
                    for kt in range(DT):
                        lhs = x_T[kt][:, j * P:(j + 1) * P]
                        nc.tensor.matmul(
                            pv[:, 0:512], lhsT=lhs, rhs=w_bf[(2, kt)][:, 0:512],
                            start=(kt == 0), stop=(kt == DT - 1))
                        nc.tensor.matmul(
                            pv[:, 512:D], lhsT=lhs, rhs=w_bf[(2, kt)][:, 512:D],
                            start=(kt == 0), stop=(kt == DT - 1))
                    vdst = v_ext[j].rearrange("p (h c) -> p h c", c=HDE)[:, :, 0:HD]
                    vsrc = pv.rearrange("p (h c) -> p h c", c=HD)
                    bvv = bv_bc.rearrange("p (h c) -> p h c", c=HD)
                    nc.vector.tensor_add(vdst, vsrc, bvv)

                for t in range(DT):
                    if t + 1 < DT:
                        next_qT, next_kT = qk_proj(t + 1)
                        next_p = s_exp(t + 1, next_qT, next_kT)
                    else:
                        next_p = None

                    # --- out^T_ext = [v | 1]^T @ P^T, one head at a time ---
                    for h in range(2):
                        gh = 2 * t + h
                        po = ops.tile([HDE, N], FP32, tag="ops", name=f"po_{t}_{h}")
                        for j in range(NT):
                            vl = v_ext[j][:, gh * HDE:(gh + 1) * HDE]
                            for c in range(NC2):
                                nc.tensor.matmul(
                                    po[:, c * 512:(c + 1) * 512],
                                    lhsT=vl,
                                    rhs=p_tiles[(j, c)][:, h * 512:(h + 1) * 512],
                                    start=(j == 0), stop=(j == NT - 1),
                                    skip_group_check=True)
                        ot = otp.tile([HDE, N], FP32, tag="ot", name=f"ot_{t}_{h}")
                        nc.vector.tensor_copy(out=ot, in_=po)
                        # transpose back to natural + normalize by denominators
                        for i in range(NT):
                            tps = qkps.tile([P, HDE], FP32, tag="qkps")
                            nc.tensor.transpose(
                                tps, ot[:, i * P:(i + 1) * P], ident[0:HDE, 0:HDE])
                            rc = rcp.tile([P, 1], FP32, tag="rc")
                            nc.vector.reciprocal(rc, tps[:, HD:HDE])
                            nc.vector.tensor_scalar_mul(
                                out_nat[i][:, gh * HD:(gh + 1) * HD],
                                tps[:, 0:HD], rc)
                            if h == 1:
                                # stream this pair's 128-column slice out now,
                                # so the kernel tail only waits on pair 5's
                                # slices instead of whole 384KB rows.
                                nc.gpsimd.dma_start(
                                    out=out_d[i * P:(i + 1) * P, t * P:(t + 1) * P],
                                    in_=out_nat[i][:, t * P:(t + 1) * P])

                    p_tiles = next_p

                    qT, kT = next_qT, next_kT

    if split_waits:
        _split_dma_waits(nc)
    return nc


_MAX_INLINE_WAITS = {"InstDMACopy": 1}
_DEFAULT_MAX_WAITS = 1


def _split_dma_waits(nc: bass.Bass) -> None:
    """walrus's instruction templates have limited semaphore-wait slots (one
    for the static-DMA pseudo, one for matmul's LDWEIGHTS, etc). Tile's sem
    assignment can attach more; hoist the excess into standalone
    InstEventSemaphore instructions on the issuing engine (sequencers execute
    in order, so the semantics are identical)."""
    for f in nc.m.functions:
        for bb in f.blocks:
            new_insts = []
            for inst in bb.instructions:
                si = getattr(inst, "sync_info", None)
                cap = _MAX_INLINE_WAITS.get(type(inst).__name__, _DEFAULT_MAX_WAITS)
                if si is not None and si.on_wait and len(si.on_wait) > cap:
                    waits = list(si.on_wait)
                    for wi, w in enumerate(waits[:-cap]):
                        ev = mybir.InstEventSemaphore(
                            name=f"{inst.name}-prewait{wi}")
                        ev.engine = inst.engine
                        ev.sync_info = mybir.SyncInfo(on_wait=[w], on_update=[])
                        new_insts.append(ev)
                    si.on_wait = waits[-cap:]
                    inst.sync_info = si
                new_insts.append(inst)
            bb.instructions[:] = new_insts


_BUILD_LOCK = threading.Lock()
_NC_CACHE: list = []


def _get_nc() -> bass.Bass:
    with _BUILD_LOCK:
        if not _NC_CACHE:
            _NC_CACHE.append(build_mha_bass())
    return _NC_CACHE[0]


def kernel(x, Wq, bq, Wk, bk, Wv, bv):
    x = np.ascontiguousarray(np.asarray(x, dtype=np.float32))
    Wq = np.ascontiguousarray(np.asarray(Wq, dtype=np.float32))
    Wk = np.ascontiguousarray(np.asarray(Wk, dtype=np.float32))
    Wv = np.ascontiguousarray(np.asarray(Wv, dtype=np.float32))
    bq = np.ascontiguousarray(np.asarray(bq, dtype=np.float32))
    bk = np.ascontiguousarray(np.asarray(bk, dtype=np.float32))
    bv = np.ascontiguousarray(np.asarray(bv, dtype=np.float32))
    assert x.shape == (B, N, D), x.shape

    nc = _get_nc()
    in_maps = [
        {"x": x[i], "Wq": Wq, "bq": bq, "Wk": Wk, "bk": bk, "Wv": Wv, "bv": bv}
        for i in range(B)
    ]
    res = run_bass_kernel_spmd(nc, in_maps, core_ids=list(range(B)))
    return np.stack([res.results[i]["out"] for i in range(B)], axis=0)
